# revision 10
# baseline (speedup 1.0000x reference)
"""DiscreteKDE kernel for 8 Trainium2 NeuronCores.

Full computation:
    Q = 64; H_I = inv(H_bandwidth)
    Z  = (idx[:,None]-idx[None,:]) @ H_I
    KW = (1/sqrt(2pi)) * exp(-0.5 * Z*Z)
    col_sums = concat([X_probs.sum(0), Y_probs.sum(0)])     # (64,)  <- 256MB read
    T  = dot(KW.sum(0), col_sums)
    out = T * ones((256,256,256))                            # 67MB write

Strategy: data-parallel over n. Each core streams its 125k-row shard of the
packed (n, 64) = [X|Y] matrix (DVE + PE split consumption), AllGathers the
per-core (64,) partial column sums, replicates the tiny (64,64) bandwidth
math (Newton-Schulz inverse), and fills 1/8 of the output.
"""

import os
import sys

import numpy as np

for _p in ("/opt/trn_rl_repo", "/root/.axon_site/_ro/trn_rl_repo"):
    if os.path.isdir(_p) and _p not in sys.path:
        sys.path.insert(0, _p)

import concourse.bacc as bacc
import concourse.bass as bass
import concourse.mybir as mybir
from concourse.bass_utils import run_bass_kernel_spmd
from concourse.tile import TileContext

# ---- problem constants (hardcoded per spec) ----
N_TOTAL = 1_000_000
FDIM = 61
HDIM = 3
Q = 64                      # FDIM + HDIM
KGRID = 256
HOUT = 3
NCORES = 8
ROWS_PER_CORE = N_TOTAL // NCORES          # 125000

# ---- tiling ----
P = 128                     # partitions
G = 64                      # rows per partition per tile
TILE_ROWS = P * G           # 8192
NT = 16                     # tiles per core
NPAD = NT * TILE_ROWS       # 131072 padded rows per core
TW = G * Q                  # 4096 f32 per partition = 16KB; tile = 2MB

OUT_TOTAL = KGRID ** HOUT                  # 16_777_216
OUT_PER_CORE = OUT_TOTAL // NCORES         # 2_097_152
FILL_W = 2048
FILL_ELEMS = P * FILL_W                    # 262_144 (1MB)
N_FILL = OUT_PER_CORE // FILL_ELEMS        # 8

NEWTON_ITERS = 13
INV_SQRT_2PI = 0.3989422804014327
LN_C = float(np.log(INV_SQRT_2PI))

F32 = mybir.dt.float32
AX = mybir.AxisListType
ALU = mybir.AluOpType
ACT_FN = mybir.ActivationFunctionType


def build_nc():
    nc = bacc.Bacc("TRN2", target_bir_lowering=False, debug=False,
                   num_devices=NCORES)

    c_in = nc.dram_tensor("c", [NPAD, Q], F32, kind="ExternalInput")
    h_in = nc.dram_tensor("h", [Q, Q], F32, kind="ExternalInput")
    out = nc.dram_tensor("o", [OUT_PER_CORE], F32, kind="ExternalOutput")

    idx = np.arange(Q, dtype=np.float64)
    d_const = nc.inline_tensor(
        (idx[:, None] - idx[None, :]).astype(np.float32), "dmat")
    i2_const = nc.inline_tensor(
        (2.0 * np.eye(Q)).astype(np.float32), "i2mat")

    cc_in = nc.dram_tensor("cc_in", [Q], F32)
    cc_out = nc.dram_tensor("cc_out", [NCORES * Q], F32, addr_space="Shared")

    with TileContext(nc) as tc:
        with (
            tc.tile_pool(name="const", bufs=1) as cpool,
            tc.tile_pool(name="stream", bufs=6) as spool,
            tc.tile_pool(name="small", bufs=2) as mpool,
            tc.tile_pool(name="acc", bufs=1, space=bass.MemorySpace.PSUM) as ppool,
            tc.tile_pool(name="psmall", bufs=2, space=bass.MemorySpace.PSUM) as pspool,
        ):
            # ---------- constants ----------
            ones_k = cpool.tile([P, 1], F32)        # lhsT for partition-reduce
            nc.vector.memset(ones_k[:], 1.0)
            ones_row = cpool.tile([1, P], F32)      # lhsT for bcast scalar->128
            nc.vector.memset(ones_row[:], 1.0)
            ones_q = cpool.tile([Q, 1], F32)        # lhsT for 64-partition reduce
            nc.vector.memset(ones_q[:], 1.0)
            ones_rq = cpool.tile([1, Q], F32)       # lhsT for bcast scalar->64
            nc.vector.memset(ones_rq[:], 1.0)
            lnc = cpool.tile([Q, 1], F32)           # exp bias = ln(1/sqrt(2pi))
            nc.vector.memset(lnc[:], LN_C)
            ones_fill = cpool.tile([P, FILL_W], F32)
            nc.gpsimd.memset(ones_fill[:], 1.0)

            # ---------- small inputs (SWDGE; keep HWDGE rings for the stream)
            a_t = cpool.tile([Q, Q], F32)
            nc.gpsimd.dma_start(a_t[:], h_in.ap())
            d_t = cpool.tile([Q, Q], F32)
            nc.gpsimd.dma_start(d_t[:], d_const.ap())
            i2_t = cpool.tile([Q, Q], F32)
            nc.gpsimd.dma_start(i2_t[:], i2_const.ap())

            # ---------- Phase B: Newton-Schulz inverse + kernel weights ------
            # Emitted FIRST so its serial PE/DVE chain runs under the stream.
            # alpha = 1 / (2*trace(A));  X0 = alpha * (2I) = I / trace(A)
            tmp_qq = mpool.tile([Q, Q], F32, tag="qq")
            nc.vector.tensor_mul(tmp_qq[:], a_t[:], i2_t[:])
            diag2 = mpool.tile([Q, 1], F32, tag="q1")
            nc.vector.tensor_reduce(diag2[:], tmp_qq[:], axis=AX.X, op=ALU.add)
            ps_tr = pspool.tile([1, 1], F32, tag="ps_small")
            nc.tensor.matmul(ps_tr[:], ones_q[:], diag2[:])
            tr2 = mpool.tile([1, 1], F32, tag="s11")
            nc.vector.tensor_copy(tr2[:], ps_tr[:])
            alpha = mpool.tile([1, 1], F32, tag="s11b")
            nc.vector.reciprocal(alpha[:], tr2[:])
            ps_a = pspool.tile([Q, 1], F32, tag="ps_small")
            nc.tensor.matmul(ps_a[:], ones_rq[:], alpha[:])
            al64 = mpool.tile([Q, 1], F32, tag="q1b")
            nc.vector.tensor_copy(al64[:], ps_a[:])
            x_cur = mpool.tile([Q, Q], F32, tag="newton")
            nc.vector.tensor_scalar_mul(x_cur[:], i2_t[:], al64[:])
            for _ in range(NEWTON_ITERS):
                ps_y = pspool.tile([Q, Q], F32, tag="ps_qq")
                nc.tensor.matmul(ps_y[:], a_t[:], x_cur[:])       # A @ X
                w_t = mpool.tile([Q, Q], F32, tag="newton_w")
                nc.vector.tensor_sub(w_t[:], i2_t[:], ps_y[:])    # 2I - AX
                ps_x = pspool.tile([Q, Q], F32, tag="ps_qq")
                nc.tensor.matmul(ps_x[:], x_cur[:], w_t[:])       # X @ W
                x_nxt = mpool.tile([Q, Q], F32, tag="newton")
                nc.vector.tensor_copy(x_nxt[:], ps_x[:])
                x_cur = x_nxt
            # Z' = D.T @ H_I = -Z ; only Z^2 is used so the sign is irrelevant
            ps_z = pspool.tile([Q, Q], F32, tag="ps_qq")
            nc.tensor.matmul(ps_z[:], d_t[:], x_cur[:])
            z2 = mpool.tile([Q, Q], F32, tag="qq")
            nc.scalar.square(z2[:], ps_z[:])
            kw = mpool.tile([Q, Q], F32, tag="qq2")
            nc.scalar.activation(kw[:], z2[:], ACT_FN.Exp,
                                 bias=lnc[:], scale=-0.5)
            ps_s = pspool.tile([1, Q], F32, tag="ps_small")
            nc.tensor.matmul(ps_s[:], ones_q[:], kw[:])           # KW.sum(0)
            s_sb = mpool.tile([1, Q], F32, tag="vec2")
            nc.vector.tensor_copy(s_sb[:], ps_s[:])

            # ---------- Phase A: stream shard, accumulate column sums --------
            # Stream consumption is split between DVE (elementwise acc) and PE
            # (ones-matmul into one PSUM bank, accumulating the 8 N=512 slices
            # of each tile on top of each other -> (g mod 8, q) partials).
            # Either engine alone (~110-120 f32/ns) barely keeps up with DMA;
            # together they are comfortably DMA-bound. DVE gets the early
            # tiles so its serial acc-chain finishes before the stream ends.
            NDVE = NT // 2
            acc = cpool.tile([P, TW], F32)
            ps_pe = ppool.tile([1, 512], F32)
            cv = c_in.ap().rearrange("(t p g) q -> t p (g q)", p=P, g=G)
            for t in range(NT):
                st = spool.tile([P, TW], F32, tag="stream")
                eng = nc.sync if t % 2 == 0 else nc.scalar
                eng.dma_start(st[:], cv[t])
                if t == 0:
                    nc.vector.tensor_copy(acc[:], st[:])
                elif t < NDVE:
                    nc.vector.tensor_add(acc[:], acc[:], st[:])
                else:
                    for b in range(TW // 512):
                        nc.tensor.matmul(
                            ps_pe[:], ones_k[:], st[:, b * 512:(b + 1) * 512],
                            start=(t == NDVE and b == 0),
                            stop=(t == NT - 1 and b == TW // 512 - 1))
            # fold DVE acc: (p, g*q) -> (p, q), then partition-reduce on PE
            acc2 = cpool.tile([P, Q], F32)
            acc_v = acc[:].rearrange("p (g q) -> p q g", g=G, q=Q)
            nc.vector.tensor_reduce(acc2[:], acc_v, axis=AX.X, op=ALU.add)
            ps2 = ppool.tile([1, Q], F32)
            nc.tensor.matmul(ps2[:], ones_k[:], acc2[:])
            # fold PE psum: (1, 8*q) -> (1, q), add both partials
            pe_fold = mpool.tile([1, Q], F32, tag="vec0")
            ps_pe_v = ps_pe[:].rearrange("p (g q) -> p q g", g=8, q=Q)
            nc.vector.tensor_reduce(pe_fold[:], ps_pe_v, axis=AX.X, op=ALU.add)
            part = mpool.tile([1, Q], F32, tag="vec")
            nc.vector.tensor_add(part[:], pe_fold[:], ps2[:])

            # ---------- Phase C: AllGather partial col sums (cheaper floor
            # than AllReduce), then fold the 8 ranks on DVE ----------
            nc.sync.dma_start(cc_in.ap(), part[:])
            nc.gpsimd.collective_compute(
                "AllGather", ALU.bypass,
                replica_groups=[list(range(NCORES))],
                ins=[cc_in.ap()], outs=[cc_out.ap()],
            )
            gath = mpool.tile([1, NCORES * Q], F32, tag="gath")
            nc.sync.dma_start(gath[:], cc_out.ap())
            gsum = mpool.tile([1, Q], F32, tag="vecg")
            gath_v = gath[:].rearrange("p (r q) -> p q r", r=NCORES, q=Q)
            nc.vector.tensor_reduce(gsum[:], gath_v, axis=AX.X, op=ALU.add)

            # ---------- Phase D: T = dot(s, gsum); fill output ----------
            prod = mpool.tile([1, Q], F32, tag="vec2b")
            nc.vector.tensor_mul(prod[:], s_sb[:], gsum[:])
            t_sc = mpool.tile([1, 1], F32, tag="s11c")
            nc.vector.tensor_reduce(t_sc[:], prod[:], axis=AX.X, op=ALU.add)
            ps_b = pspool.tile([P, 1], F32, tag="ps_small")
            nc.tensor.matmul(ps_b[:], ones_row[:], t_sc[:])       # bcast->(128,1)
            tb = mpool.tile([P, 1], F32, tag="q1c")
            nc.vector.tensor_copy(tb[:], ps_b[:])
            fill = spool.tile([P, FILL_W], F32, tag="fill")
            nc.vector.tensor_scalar_mul(fill[:], ones_fill[:], tb[:])
            # fill: one broadcast DMA per HWDGE ring (sync + scalar), each 4MB
            half = N_FILL // 2
            ovh = out.ap().rearrange("(h j p f) -> h p j f", h=2, p=P, f=FILL_W)
            fill_b = fill[:].unsqueeze(1).broadcast_to([P, half, FILL_W])
            nc.sync.dma_start(ovh[0], fill_b)
            nc.scalar.dma_start(ovh[1], fill_b)

    nc.compile()
    return nc


_NC_CACHE = None


def _get_nc():
    global _NC_CACHE
    if _NC_CACHE is None:
        _NC_CACHE = build_nc()
    return _NC_CACHE


def run(X_probs, Y_probs, H_bandwidth, trace=False, trace_kwargs=None):
    X = np.asarray(X_probs, dtype=np.float32).reshape(NCORES, ROWS_PER_CORE, FDIM)
    Y = np.asarray(Y_probs, dtype=np.float32).reshape(NCORES, ROWS_PER_CORE, HDIM)
    H = np.ascontiguousarray(np.asarray(H_bandwidth, dtype=np.float32))

    C = np.zeros((NCORES, NPAD, Q), dtype=np.float32)
    C[:, :ROWS_PER_CORE, :FDIM] = X
    C[:, :ROWS_PER_CORE, FDIM:] = Y

    nc = _get_nc()
    in_maps = [{"c": C[i], "h": H} for i in range(NCORES)]
    res = run_bass_kernel_spmd(nc, in_maps, list(range(NCORES)),
                               trace=trace, **(trace_kwargs or {}))
    full = np.concatenate([res.results[i]["o"] for i in range(NCORES)])
    return full.reshape((KGRID,) * HOUT), res


def kernel(X_probs, Y_probs, H_bandwidth, K, H_out):
    assert int(K) == KGRID and int(H_out) == HOUT
    out, _ = run(X_probs, Y_probs, H_bandwidth, trace=False)
    return out


# revision 12
# speedup vs baseline: 1.0893x; 1.0893x over previous
"""DiscreteKDE kernel for 8 Trainium2 NeuronCores.

Full computation:
    Q = 64; H_I = inv(H_bandwidth)
    Z  = (idx[:,None]-idx[None,:]) @ H_I
    KW = (1/sqrt(2pi)) * exp(-0.5 * Z*Z)
    col_sums = concat([X_probs.sum(0), Y_probs.sum(0)])     # (64,)  <- 256MB read
    T  = dot(KW.sum(0), col_sums)
    out = T * ones((256,256,256))                            # 67MB write

Strategy: data-parallel over n. Each core streams its 125k-row shard of the
packed (n, 64) = [X|Y] matrix, accumulating tiles elementwise on DVE (the
stream is DMA-bound at ~350GB/s), AllGathers the per-core (64,) partial
column sums, replicates the tiny (64,64) bandwidth math (Newton-Schulz
inverse on PE/ACT/GpSimd, hidden under the stream), and fills 1/8 of the
output.
"""

import os
import sys

import numpy as np

for _p in ("/opt/trn_rl_repo", "/root/.axon_site/_ro/trn_rl_repo"):
    if os.path.isdir(_p) and _p not in sys.path:
        sys.path.insert(0, _p)

import concourse.bacc as bacc
import concourse.bass as bass
import concourse.mybir as mybir
from concourse.bass_utils import run_bass_kernel_spmd
from concourse.tile import TileContext

# ---- problem constants (hardcoded per spec) ----
N_TOTAL = 1_000_000
FDIM = 61
HDIM = 3
Q = 64                      # FDIM + HDIM
KGRID = 256
HOUT = 3
NCORES = 8
ROWS_PER_CORE = N_TOTAL // NCORES          # 125000

# ---- tiling ----
P = 128                     # partitions
G = 64                      # rows per partition per full tile
TILE_ROWS = P * G           # 8192
NFULL = ROWS_PER_CORE // TILE_ROWS         # 15 full tiles
TAIL_ROWS = ROWS_PER_CORE - NFULL * TILE_ROWS   # 2120
TAIL_P = 106                # 2120 = 106 partitions * 20 rows
TAIL_G = TAIL_ROWS // TAIL_P               # 20
TW = G * Q                  # 4096 f32 per partition = 16KB; tile = 2MB
NT = NFULL + 1

OUT_TOTAL = KGRID ** HOUT                  # 16_777_216
OUT_PER_CORE = OUT_TOTAL // NCORES         # 2_097_152
FILL_W = 2048
N_FILL = OUT_PER_CORE // (P * FILL_W)      # 8

NEWTON_ITERS = 13
INV_SQRT_2PI = 0.3989422804014327
LN_C = float(np.log(INV_SQRT_2PI))

F32 = mybir.dt.float32
AX = mybir.AxisListType
ALU = mybir.AluOpType
ACT_FN = mybir.ActivationFunctionType


def build_nc():
    nc = bacc.Bacc("TRN2", target_bir_lowering=False, debug=False,
                   num_devices=NCORES)

    c_in = nc.dram_tensor("c", [ROWS_PER_CORE, Q], F32, kind="ExternalInput")
    h_in = nc.dram_tensor("h", [Q, Q], F32, kind="ExternalInput")
    out = nc.dram_tensor("o", [OUT_PER_CORE], F32, kind="ExternalOutput")

    idx = np.arange(Q, dtype=np.float64)
    d_const = nc.inline_tensor(
        (idx[:, None] - idx[None, :]).astype(np.float32), "dmat")
    i2_const = nc.inline_tensor(
        (2.0 * np.eye(Q)).astype(np.float32), "i2mat")

    cc_in = nc.dram_tensor("cc_in", [Q], F32)
    cc_out = nc.dram_tensor("cc_out", [NCORES * Q], F32, addr_space="Shared")

    with TileContext(nc) as tc:
        with (
            tc.tile_pool(name="const", bufs=1) as cpool,
            tc.tile_pool(name="stream", bufs=6) as spool,
            tc.tile_pool(name="small", bufs=2) as mpool,
            tc.tile_pool(name="accp", bufs=1, space=bass.MemorySpace.PSUM) as ppool,
            tc.tile_pool(name="psmall", bufs=2, space=bass.MemorySpace.PSUM) as pspool,
        ):
            # ---------- Phase A streams: emit first so sync/scalar rings
            # start the big DMAs immediately ----------
            cv = c_in.ap()[:NFULL * TILE_ROWS, :].rearrange(
                "(t p g) q -> t p (g q)", p=P, g=G)
            tail_v = c_in.ap()[NFULL * TILE_ROWS:, :].rearrange(
                "(p g) q -> p (g q)", p=TAIL_P, g=TAIL_G)
            tiles = []
            for t in range(NT):
                st = spool.tile([P, TW], F32, tag="stream")
                eng = nc.sync if t % 2 == 0 else nc.scalar
                if t < NFULL:
                    eng.dma_start(st[:], cv[t])
                else:
                    eng.dma_start(st[:TAIL_P, :TAIL_G * Q], tail_v)
                tiles.append(st)

            # ---------- constants ----------
            ones_k = cpool.tile([P, 1], F32)        # lhsT for partition-reduce
            nc.vector.memset(ones_k[:], 1.0)
            ones_row = cpool.tile([1, P], F32)      # lhsT for bcast scalar->128
            nc.vector.memset(ones_row[:], 1.0)
            ones_q = cpool.tile([Q, 1], F32)        # lhsT for 64-partition reduce
            nc.vector.memset(ones_q[:], 1.0)
            ones_rq = cpool.tile([1, Q], F32)       # lhsT for bcast scalar->64
            nc.vector.memset(ones_rq[:], 1.0)
            lnc = cpool.tile([Q, 1], F32)           # exp bias = ln(1/sqrt(2pi))
            nc.vector.memset(lnc[:], LN_C)
            ones_fill = cpool.tile([P, FILL_W], F32)
            nc.gpsimd.memset(ones_fill[:], 1.0)

            # ---------- small inputs (SWDGE; keep HWDGE rings for the stream)
            a_t = cpool.tile([Q, Q], F32)
            nc.gpsimd.dma_start(a_t[:], h_in.ap())
            d_t = cpool.tile([Q, Q], F32)
            nc.gpsimd.dma_start(d_t[:], d_const.ap())
            i2_t = cpool.tile([Q, Q], F32)
            nc.gpsimd.dma_start(i2_t[:], i2_const.ap())

            # ---------- Phase B: Newton-Schulz inverse + kernel weights ------
            # Runs on PE + ACT + GpSimd only; DVE is reserved for the stream
            # accumulation. The serial chain hides under the DMA stream.
            # alpha = 1 / (2*trace(A));  X0 = alpha * (2I) = I / trace(A)
            tmp_qq = mpool.tile([Q, Q], F32, tag="qq")
            nc.gpsimd.tensor_mul(tmp_qq[:], a_t[:], i2_t[:])
            diag2 = mpool.tile([Q, 1], F32, tag="q1")
            nc.vector.tensor_reduce(diag2[:], tmp_qq[:], axis=AX.X, op=ALU.add)
            ps_tr = pspool.tile([1, 1], F32, tag="ps_small")
            nc.tensor.matmul(ps_tr[:], ones_q[:], diag2[:])
            tr2 = mpool.tile([1, 1], F32, tag="s11")
            nc.vector.reciprocal(tr2[:], ps_tr[:])
            ps_a = pspool.tile([Q, 1], F32, tag="ps_small")
            nc.tensor.matmul(ps_a[:], ones_rq[:], tr2[:])
            al64 = mpool.tile([Q, 1], F32, tag="q1b")
            nc.scalar.activation(al64[:], ps_a[:], ACT_FN.Copy)
            x_cur = mpool.tile([Q, Q], F32, tag="newton")
            nc.gpsimd.tensor_scalar_mul(x_cur[:], i2_t[:], al64[:])
            for _ in range(NEWTON_ITERS):
                ps_y = pspool.tile([Q, Q], F32, tag="ps_qq")
                nc.tensor.matmul(ps_y[:], a_t[:], x_cur[:])       # A @ X
                y_sb = mpool.tile([Q, Q], F32, tag="newton_y")
                nc.scalar.activation(y_sb[:], ps_y[:], ACT_FN.Copy)
                w_t = mpool.tile([Q, Q], F32, tag="newton_w")
                nc.gpsimd.tensor_sub(w_t[:], i2_t[:], y_sb[:])    # 2I - AX
                ps_x = pspool.tile([Q, Q], F32, tag="ps_qq")
                nc.tensor.matmul(ps_x[:], x_cur[:], w_t[:])       # X @ W
                x_nxt = mpool.tile([Q, Q], F32, tag="newton")
                nc.scalar.activation(x_nxt[:], ps_x[:], ACT_FN.Copy)
                x_cur = x_nxt
            # Z' = D.T @ H_I = -Z ; only Z^2 is used so the sign is irrelevant
            ps_z = pspool.tile([Q, Q], F32, tag="ps_qq")
            nc.tensor.matmul(ps_z[:], d_t[:], x_cur[:])
            z2 = mpool.tile([Q, Q], F32, tag="qq")
            nc.scalar.square(z2[:], ps_z[:])
            kw = mpool.tile([Q, Q], F32, tag="qq2")
            nc.scalar.activation(kw[:], z2[:], ACT_FN.Exp,
                                 bias=lnc[:], scale=-0.5)
            ps_s = pspool.tile([1, Q], F32, tag="ps_small")
            nc.tensor.matmul(ps_s[:], ones_q[:], kw[:])           # KW.sum(0)
            s_sb = mpool.tile([1, Q], F32, tag="vec2")
            nc.scalar.activation(s_sb[:], ps_s[:], ACT_FN.Copy)

            # ---------- Phase A compute: DVE elementwise accumulation --------
            acc = cpool.tile([P, TW], F32)
            for t in range(NT):
                st = tiles[t]
                if t == 0:
                    nc.vector.tensor_copy(acc[:], st[:])
                elif t < NFULL:
                    nc.vector.tensor_add(acc[:], acc[:], st[:])
                else:
                    nc.vector.tensor_add(acc[:TAIL_P, :TAIL_G * Q],
                                         acc[:TAIL_P, :TAIL_G * Q],
                                         st[:TAIL_P, :TAIL_G * Q])
            # fold the G row-groups within each partition: (p, g*q) -> (p, q)
            acc2 = cpool.tile([P, Q], F32)
            acc_v = acc[:].rearrange("p (g q) -> p q g", g=G, q=Q)
            nc.vector.tensor_reduce(acc2[:], acc_v, axis=AX.X, op=ALU.add)
            # partition reduction on PE, then out to SBUF via ACT
            ps2 = ppool.tile([1, Q], F32)
            nc.tensor.matmul(ps2[:], ones_k[:], acc2[:])
            part = mpool.tile([1, Q], F32, tag="vec")
            nc.scalar.activation(part[:], ps2[:], ACT_FN.Copy)

            # ---------- Phase C: AllGather partial col sums (cheaper floor
            # than AllReduce), then fold the 8 ranks on DVE ----------
            nc.sync.dma_start(cc_in.ap(), part[:])
            nc.gpsimd.collective_compute(
                "AllGather", ALU.bypass,
                replica_groups=[list(range(NCORES))],
                ins=[cc_in.ap()], outs=[cc_out.ap()],
            )
            gath = mpool.tile([1, NCORES * Q], F32, tag="gath")
            nc.sync.dma_start(gath[:], cc_out.ap())
            gsum = mpool.tile([1, Q], F32, tag="vecg")
            gath_v = gath[:].rearrange("p (r q) -> p q r", r=NCORES, q=Q)
            nc.vector.tensor_reduce(gsum[:], gath_v, axis=AX.X, op=ALU.add)

            # ---------- Phase D: T = dot(s, gsum); fill output ----------
            prod = mpool.tile([1, Q], F32, tag="vec2b")
            nc.vector.tensor_mul(prod[:], s_sb[:], gsum[:])
            t_sc = mpool.tile([1, 1], F32, tag="s11c")
            nc.vector.tensor_reduce(t_sc[:], prod[:], axis=AX.X, op=ALU.add)
            ps_b = pspool.tile([P, 1], F32, tag="ps_small")
            nc.tensor.matmul(ps_b[:], ones_row[:], t_sc[:])       # bcast->(128,1)
            tb = mpool.tile([P, 1], F32, tag="q1c")
            nc.scalar.activation(tb[:], ps_b[:], ACT_FN.Copy)
            fill = spool.tile([P, FILL_W], F32, tag="fill")
            nc.vector.tensor_scalar_mul(fill[:], ones_fill[:], tb[:])
            # fill: one broadcast DMA per HWDGE ring (sync + scalar), each 4MB
            half = N_FILL // 2
            ovh = out.ap().rearrange("(h j p f) -> h p j f", h=2, p=P, f=FILL_W)
            fill_b = fill[:].unsqueeze(1).broadcast_to([P, half, FILL_W])
            nc.sync.dma_start(ovh[0], fill_b)
            nc.scalar.dma_start(ovh[1], fill_b)

    nc.compile()
    return nc


_NC_CACHE = None


def _get_nc():
    global _NC_CACHE
    if _NC_CACHE is None:
        _NC_CACHE = build_nc()
    return _NC_CACHE


def run(X_probs, Y_probs, H_bandwidth, trace=False, trace_kwargs=None):
    X = np.asarray(X_probs, dtype=np.float32).reshape(NCORES, ROWS_PER_CORE, FDIM)
    Y = np.asarray(Y_probs, dtype=np.float32).reshape(NCORES, ROWS_PER_CORE, HDIM)
    H = np.ascontiguousarray(np.asarray(H_bandwidth, dtype=np.float32))

    C = np.empty((NCORES, ROWS_PER_CORE, Q), dtype=np.float32)
    C[:, :, :FDIM] = X
    C[:, :, FDIM:] = Y

    nc = _get_nc()
    in_maps = [{"c": C[i], "h": H} for i in range(NCORES)]
    res = run_bass_kernel_spmd(nc, in_maps, list(range(NCORES)),
                               trace=trace, **(trace_kwargs or {}))
    full = np.concatenate([res.results[i]["o"] for i in range(NCORES)])
    return full.reshape((KGRID,) * HOUT), res


def kernel(X_probs, Y_probs, H_bandwidth, K, H_out):
    assert int(K) == KGRID and int(H_out) == HOUT
    out, _ = run(X_probs, Y_probs, H_bandwidth, trace=False)
    return out
